# revision 5
# baseline (speedup 1.0000x reference)
"""Trainium2 Bass kernel for nn_CrossAttentionFusion.

Math: softmax over kv_len==1 is identically 1.0, so the attention output is
v broadcast over the N (patch) axis and the whole module reduces to

    out[b, n, :] = cnn[b] @ (Wkv[:, C:] @ Wp) + bp        (independent of n)

W_eff = Wkv[:, C:] @ Wp is a weight-only constant, folded on the host.

Strategy: COLUMN-parallel over the C=768 output columns across 8 NeuronCores
(96 columns per core, full batch on every core), fp16 end-to-end on device.
Per core the inputs are tiny (~0.95 MB fp16) and the output write dominates:
64*576*96 fp16 = 7.08 MB. The harness gate is rel_err < 2e-2; fp16 adds ~4e-4.

v2 pipeline (vs v1's 4-group one-hot fan-out): the batch fan-out happens
INSIDE the projection matmul. Each batch's cnn column is duplicated onto 2 of
the 128 lhsT columns, so the 17 accumulating matmuls directly produce
ps[p, c] = y[p//2, c] on all 128 partitions (partition p owns the contiguous
288-row half n in [(p%2)*288, ...) of batch p//2's 576 output rows). Then:
 1. One fused input DMA (wc = interleaved lhsT/rhs chunks + bias chunk),
    split in 3 pieces across both HWDGE rings so matmuls start early.
 2. 17 accumulating matmuls -> ps[128, 96] (bias via 17th ones/bp chunk).
 3. One PSUM->SBUF fp16 cast + log2 widen copies -> bc[128, 36*96]
    (row replicated 36x along the free axis).
 4. Two DMAs (one per HWDGE ring, j-halves) write the full 7.08 MB with
    6912-B descriptors (stride-0-source j broadcast repeats each partition's
    36 SBUF rows to its 288 dst rows).
"""

import sys

sys.path.insert(0, "/opt/trn_rl_repo")

import numpy as np

import concourse.bass as bass
import concourse.mybir as mybir
from concourse import bacc
from concourse.bass_utils import run_bass_kernel_spmd
from concourse.tile import TileContext

F32 = mybir.dt.float32
F16 = mybir.dt.float16

NCORES = 8
B, N, C, CNN = 64, 576, 768, 2048
CPC = C // NCORES  # 96 output columns per core
KC = CNN // 128 + 1  # 16 contraction chunks + 1 bias chunk
CHUNK = 128 + CPC  # per-chunk cols in the fused wc input: 128 lhsT + 96 rhs
REP = 36  # SBUF replication depth: 6912-B DMA descriptors
JPP = (B * N) // 128  # 288 dst rows per partition
JB = JPP // REP  # 8 stride-0 j repeats in the DMA


def _build_bass():
    nc = bacc.Bacc(None, target_bir_lowering=False, debug=False, num_devices=NCORES)

    x_wc = nc.declare_dram_parameter("wc", [128, KC * CHUNK], F16, isOutput=False)
    yo = nc.declare_dram_parameter("out", [B * N, CPC], F16, isOutput=True)

    with TileContext(nc) as tc:
        with (
            tc.tile_pool(name="singles", bufs=1) as singles,
            tc.tile_pool(name="psum", bufs=1, space="PSUM") as psum,
        ):
            # fused input, split loads across both rings so matmuls overlap
            # the tail of the transfer; tiny first piece so MM0 starts early
            wc_t = singles.tile([128, KC * CHUNK], F16, tag="wc")
            for (lo, hi), eng in (
                ((0, 2), nc.sync),
                ((2, 6), nc.scalar),
                ((6, 11), nc.sync),
                ((11, KC), nc.scalar),
            ):
                eng.dma_start(
                    out=wc_t[:, lo * CHUNK : hi * CHUNK],
                    in_=x_wc[:, lo * CHUNK : hi * CHUNK],
                )

            # Projection with fan-out built into lhsT: ps[p, c] = y[p//2, c]
            ps = psum.tile([128, 512], F32, tag="ps")
            for k in range(KC):
                nc.tensor.matmul(
                    ps[:, 0:CPC],
                    wc_t[:, k * CHUNK : k * CHUNK + 128],
                    wc_t[:, k * CHUNK + 128 : (k + 1) * CHUNK],
                    start=(k == 0),
                    stop=(k == KC - 1),
                )

            # PSUM->SBUF fp16 cast, then log2 doubling copies. An early DMA
            # with REP=12 (2304-B descriptors) streams rows [0:36) as soon as
            # 12 copies exist; the remaining widen to REP=36 overlaps it, and
            # two big DMAs (6912-B descriptors) cover rows [36:288).
            bc = singles.tile([128, REP * CPC], F16, tag="bc")
            nc.vector.tensor_copy(bc[:, 0:CPC], ps[:, 0:CPC])
            for w, n in ((CPC, CPC), (2 * CPC, 2 * CPC), (4 * CPC, 4 * CPC), (8 * CPC, 4 * CPC)):
                nc.vector.tensor_copy(bc[:, w : w + n], bc[:, 0:n])

            # per-partition row view of the output: partition p owns dst rows
            # [p*288, (p+1)*288) = the contiguous half n-range of batch p//2
            rows = yo.rearrange("(p n) c -> p n c", p=128)

            def out_dma(eng, r0, r1, rep):
                jb = (r1 - r0) // rep
                dst = rows[:, r0:r1, :].rearrange("p (j r) c -> p j (r c)", r=rep)
                src = (
                    bc[:, 0 : rep * CPC]
                    .unsqueeze(1)
                    .broadcast_to((128, jb, rep * CPC))
                )
                eng.dma_start(out=dst, in_=src)

            out_dma(nc.sync, 0, 36, 12)  # early: needs only bc[:, 0:1152]

            nc.vector.tensor_copy(bc[:, 12 * CPC : 24 * CPC], bc[:, 0 : 12 * CPC])
            nc.vector.tensor_copy(bc[:, 24 * CPC : 36 * CPC], bc[:, 0 : 12 * CPC])

            out_dma(nc.scalar, 36, 180, REP)
            out_dma(nc.sync, 180, 288, REP)

    nc.compile()
    return nc


_NC = None


def _get_nc():
    global _NC
    if _NC is None:
        _NC = _build_bass()
    return _NC


def _prepare_in_maps(image_patches, cnn_feature_vector, Wq, Wkv, Wp, bp):
    Weff = (np.ascontiguousarray(Wkv[:, C:]) @ Wp).astype(np.float16)  # (2048, 768)
    # lhsT chunks: [128 contraction rows, 128 out partitions]; out partition
    # p carries batch p//2, so each batch's cnn column appears twice
    cnnT2 = np.repeat(
        cnn_feature_vector.astype(np.float16).T.reshape(KC - 1, 128, B), 2, axis=2
    )  # (16, 128, 128)

    in_maps = []
    for core in range(NCORES):
        c0 = core * CPC
        wc = np.zeros((128, KC * CHUNK), dtype=np.float16)
        for k in range(KC - 1):
            wc[:, k * CHUNK : k * CHUNK + 128] = cnnT2[k]
            wc[:, k * CHUNK + 128 : (k + 1) * CHUNK] = Weff[
                k * 128 : (k + 1) * 128, c0 : c0 + CPC
            ]
        # bias chunk: ones row in lhsT x bp row in rhs
        wc[0, (KC - 1) * CHUNK : (KC - 1) * CHUNK + 128] = 1.0
        wc[0, (KC - 1) * CHUNK + 128 : KC * CHUNK] = bp[c0 : c0 + CPC]
        in_maps.append({"wc": wc})
    return in_maps


def _assemble(res):
    out = np.empty((B, N, C), dtype=np.float32)
    for i in range(NCORES):
        out[:, :, i * CPC : (i + 1) * CPC] = res.results[i]["out"].reshape(B, N, CPC)
    return out


def kernel(**inputs) -> np.ndarray:
    inputs = {k: np.asarray(v) for k, v in inputs.items()}
    nc = _get_nc()
    in_maps = _prepare_in_maps(**inputs)
    res = run_bass_kernel_spmd(nc, in_maps, core_ids=list(range(NCORES)))
    return _assemble(res)


def kernel_traced(**inputs):
    """kernel() + HW profile; returns (output, BassKernelResults)."""
    inputs = {k: np.asarray(v) for k, v in inputs.items()}
    nc = _get_nc()
    in_maps = _prepare_in_maps(**inputs)
    res = run_bass_kernel_spmd(
        nc,
        in_maps,
        core_ids=list(range(NCORES)),
        trace=True,
        trace_cores=list(range(NCORES)),
    )
    return _assemble(res), res
